# revision 26
# baseline (speedup 1.0000x reference)
"""Trainium2 Bass kernel for nn_PersistentObserver (GNN message passing).

Math (per batch item b, N=256 nodes):
  node_emb = relu(relu(obs@W1+b1)@W2+b2)            [N,256]
  upd      = node_emb@Wu+bu                         [N,128]
  lat      = GRUCell(upd, latent)                   [N,128]
  pair_ij  = [lat_i, lat_j, |lat_i-lat_j|]          [N,N,384]
  edge     = relu(pair@We1+be1)@We2+be2, diag=-8    [N,N]
  op       = relu(pair@Wo1+bo1)@Wo2+bo2             [N,N,8]
  next     = relu([lat,node_emb,q_emb]@Wn1+bn1)@Wn2+bn2  [N,1]

Key factorization: pair@W1 = A_i + B_j + |lat_i-lat_j|@W1a where
A = lat@W1_left, B = lat@W1_right depend on one index only. Only the
abs-diff term needs the N^2 matmul.

Sharding: 8 cores = 4 batches x 2 row-halves. Odd cores receive inputs
rolled by -128 along the node axis (the computation is permutation
equivariant), so every core runs the identical program computing rows
0..127; the host un-rolls the j axis on gather.

Layout: transposed ("T"): features on partitions, nodes on the free
axis. Hidden H=256 split in two partition halves. Inner loop handles
2 rows per iteration (free dim 512). Second-layer outputs accumulate
in PSUM across iterations using phase-shifted weight matrices (the w2
column placed at the iteration's output row, zeros elsewhere), so the
PSUM->SBUF->DRAM drain happens once per 16 (edge) / 4 (op) iterations;
output biases are folded into the drain copies as per-partition scalars.
B-term handling is asymmetric: the edge head accumulates B on the PE
(second matmul into PSUM); the op head adds a precomputed SBUF copy of B
via one DVE tensor_tensor, which lets its relu epilogue run as 2x-mode
SBUF-source tensor_scalars instead of 1x PSUM-source ops.
"""
import numpy as np
from contextlib import ExitStack

import concourse.bass as bass
import concourse.tile as tile
from concourse import bacc, mybir
from concourse.bass_utils import run_bass_kernel_spmd

F32 = mybir.dt.float32
F32R = mybir.dt.float32r
F16 = mybir.dt.float16
I32 = mybir.dt.int32
A = mybir.AluOpType
AF = mybir.ActivationFunctionType

B, N, OBS, QDIM = 4, 256, 64, 32
H, D = 256, 128
NOPS = 8
QE = 64          # H // 4
NI = 128         # i-rows per core
N_CORES = 8
EBLK = 16        # edge accumulation block (iterations)
OBLK = 4         # op accumulation block

_NC_CACHE = {}


def _build():
    nc = bacc.Bacc("TRN2", target_bir_lowering=False, debug=False,
                   num_devices=N_CORES)

    di = {}
    def inp(name, shape):
        di[name] = nc.dram_tensor(name, list(shape), F32, kind="ExternalInput").ap()
        return di[name]

    inp("enc_w1", (OBS, H)); inp("enc_b1", (H,))
    inp("enc_w2", (H, H)); inp("enc_b2", (H,))
    inp("upd_w", (H, D)); inp("upd_b", (D,))
    inp("gru_wi", (D, 3 * D)); inp("gru_bi", (3 * D,))
    inp("gru_wh", (D, 3 * D)); inp("gru_bh", (3 * D,))
    inp("eh_w1", (3 * D, H)); inp("eh_b1", (H,))
    inp("eh_w2", (H, 1)); inp("eh_b2", (1,))
    inp("oh_w1", (3 * D, H)); inp("oh_b1", (H,))
    inp("oh_w2", (H, NOPS)); inp("oh_b2", (NOPS,))
    inp("q_w", (QDIM, QE)); inp("q_b", (QE,))
    inp("nh_w1", (D + H + QE, H)); inp("nh_b1", (H,))
    inp("nh_w2", (H, 1)); inp("nh_b2", (1,))
    inp("obs_t", (OBS, N)); inp("latent_t", (D, N)); inp("query_t", (QDIM, N))
    inp("ew2sh_in", (128, 2 * EBLK * EBLK)); inp("ow2sh_in", (128, 2 * OBLK * 32))
    inp("col_pack", (128, 2))

    edge_d = nc.dram_tensor("edge_out", [NI, N], F32, kind="ExternalOutput").ap()
    op_d = nc.dram_tensor("op_out", [NI * NOPS, N], F32, kind="ExternalOutput").ap()
    np_d = nc.dram_tensor("np_out", [N], F32, kind="ExternalOutput").ap()

    with tile.TileContext(nc) as tc, ExitStack() as ctx:
        cst = ctx.enter_context(tc.tile_pool(name="cst", bufs=1))
        act = ctx.enter_context(tc.tile_pool(name="act", bufs=1))
        wrk = ctx.enter_context(tc.tile_pool(name="wrk", bufs=3))
        ps1 = ctx.enter_context(tc.tile_pool(name="ps1", bufs=1, space="PSUM"))
        ps2 = ctx.enter_context(tc.tile_pool(name="ps2", bufs=2, space="PSUM"))

        # ---------------- weight / bias loads ----------------
        def load(name, view, shape, dt=F32, eng=None):
            t = cst.tile(list(shape), dt, tag=name)
            src = view if dt == F32 else view.bitcast(F32R)
            (eng or nc.sync).dma_start(t[:], src)
            return t

        # critical-path loads first: activations + encoder/GRU weights
        obsT = load("obsT", di["obs_t"][:], (OBS, N))
        latT = load("latT", di["latent_t"][:], (D, N))
        qT = load("qT", di["query_t"][:], (QDIM, N))
        ew1 = load("ew1", di["enc_w1"][:], (OBS, H))
        eb1 = load("eb1", di["enc_b1"].rearrange("(s p) -> p s", s=2), (128, 2))
        ew2 = load("ew2", di["enc_w2"].rearrange("(kh kl) m -> kl kh m", kh=2), (128, 512))
        eb2 = load("eb2", di["enc_b2"].rearrange("(s p) -> p s", s=2), (128, 2))
        uw = load("uw", di["upd_w"].rearrange("(kh kl) m -> kl kh m", kh=2), (128, 256))
        ub = load("ub", di["upd_b"].rearrange("(p o) -> p o", o=1), (128, 1))
        gwi = load("gwi", di["gru_wi"][:], (128, 384))
        gbi = load("gbi", di["gru_bi"].rearrange("(s p) -> p s", s=3), (128, 3))
        gwh = load("gwh", di["gru_wh"][:], (128, 384))
        gbh = load("gbh", di["gru_bh"].rearrange("(s p) -> p s", s=3), (128, 3))
        hw1e = load("hw1e", di["eh_w1"].rearrange("(s kl) m -> kl s m", s=3), (128, 768), F32R, nc.gpsimd)
        hb1e = load("hb1e", di["eh_b1"].rearrange("(s p) -> p s", s=2), (128, 2))
        hw1o = load("hw1o", di["oh_w1"].rearrange("(s kl) m -> kl s m", s=3), (128, 768), F32R, nc.gpsimd)
        hb1o = load("hb1o", di["oh_b1"].rearrange("(s p) -> p s", s=2), (128, 2))
        qw = load("qw", di["q_w"][:], (QDIM, QE))
        qb = load("qb", di["q_b"].rearrange("(p o) -> p o", o=1), (QE, 1))
        nw1a = load("nw1a", di["nh_w1"][0:384, :].rearrange("(s kl) m -> kl s m", s=3), (128, 768), F32, nc.gpsimd)
        nw1b = load("nw1b", di["nh_w1"][384:448, :], (QE, H))
        nb1 = load("nb1", di["nh_b1"].rearrange("(s p) -> p s", s=2), (128, 2))
        nw2 = load("nw2", di["nh_w2"].rearrange("(s kl) m -> kl s m", s=2), (128, 2))
        nb2 = load("nb2", di["nh_b2"].rearrange("(p o) -> p o", o=1), (1, 1))

        # phase-shifted second-layer weights (host-packed) + drain biases
        ew2sh = load("ew2sh", di["ew2sh_in"][:], (128, 2 * EBLK * EBLK), F32R, nc.gpsimd)
        ow2sh = load("ow2sh", di["ow2sh_in"][:], (128, 2 * OBLK * 32), F32R, nc.gpsimd)
        colp = load("colp", di["col_pack"][:], (128, 2))

        # ---------------- per-batch precompute ----------------
        PC_TAGS = ["he0", "he1", "ho0", "ho1"]
        pc_i = [0]
        def pc_psum(p_dim, f_dim):
            t = ps1.tile([p_dim, f_dim], F32, tag=PC_TAGS[pc_i[0] % 4])
            pc_i[0] += 1
            return t

        # encoder layer 1: h1T[hh] = relu(W1[:,hh]^T @ obsT + b1)
        h1T = act.tile([128, 512], F32, tag="h1T")
        for hh in range(2):
            ps = pc_psum(128, N)
            nc.tensor.matmul(ps[:], ew1[:, hh * 128:(hh + 1) * 128], obsT[:],
                             start=True, stop=True)
            nc.scalar.activation(h1T[:, hh * 256:(hh + 1) * 256], ps[:],
                                 AF.Relu, bias=eb1[:, hh:hh + 1])
        # encoder layer 2
        nembT = act.tile([128, 512], F32, tag="nembT")
        for hh in range(2):
            ps = pc_psum(128, N)
            for kh in range(2):
                nc.tensor.matmul(ps[:], ew2[:, kh * 256 + hh * 128: kh * 256 + (hh + 1) * 128],
                                 h1T[:, kh * 256:(kh + 1) * 256],
                                 start=(kh == 0), stop=(kh == 1))
            nc.scalar.activation(nembT[:, hh * 256:(hh + 1) * 256], ps[:],
                                 AF.Relu, bias=eb2[:, hh:hh + 1])
        # upd head
        updT = act.tile([D, N], F32, tag="updT")
        ps = pc_psum(D, N)
        for kh in range(2):
            nc.tensor.matmul(ps[:], uw[:, kh * 128:(kh + 1) * 128],
                             nembT[:, kh * 256:(kh + 1) * 256],
                             start=(kh == 0), stop=(kh == 1))
        nc.scalar.activation(updT[:], ps[:], AF.Identity, bias=ub[:])
        # GRU gates
        giT = act.tile([D, 768], F32, tag="giT")
        ghT = act.tile([D, 768], F32, tag="ghT")
        for g in range(3):
            ps = pc_psum(D, N)
            nc.tensor.matmul(ps[:], gwi[:, g * 128:(g + 1) * 128], updT[:],
                             start=True, stop=True)
            nc.scalar.activation(giT[:, g * 256:(g + 1) * 256], ps[:],
                                 AF.Identity, bias=gbi[:, g:g + 1])
            ps = pc_psum(D, N)
            nc.tensor.matmul(ps[:], gwh[:, g * 128:(g + 1) * 128], latT[:],
                             start=True, stop=True)
            nc.scalar.activation(ghT[:, g * 256:(g + 1) * 256], ps[:],
                                 AF.Identity, bias=gbh[:, g:g + 1])
        rT = act.tile([D, N], F32, tag="rT")
        nc.vector.tensor_add(rT[:], giT[:, 0:256], ghT[:, 0:256])
        nc.scalar.activation(rT[:], rT[:], AF.Sigmoid)
        zT = act.tile([D, N], F32, tag="zT")
        nc.vector.tensor_add(zT[:], giT[:, 256:512], ghT[:, 256:512])
        nc.scalar.activation(zT[:], zT[:], AF.Sigmoid)
        nT = act.tile([D, N], F32, tag="nT")
        nc.vector.tensor_mul(nT[:], rT[:], ghT[:, 512:768])
        nc.vector.tensor_add(nT[:], nT[:], giT[:, 512:768])
        nc.scalar.activation(nT[:], nT[:], AF.Tanh)
        # latn = n + z*(lat - n)
        latn = act.tile([D, N], F32, tag="latn")
        nc.vector.tensor_sub(latn[:], latT[:], nT[:])
        nc.vector.tensor_mul(latn[:], zT[:], latn[:])
        nc.vector.tensor_add(latn[:], latn[:], nT[:])
        # duplicated f32r copy [latn | latn] for the per-i B accumulation
        latn2 = act.tile([D, 512], F32R, tag="latn2")
        nc.vector.tensor_scalar(latn2[:, 0:256], latn[:], 0.0, None, op0=A.add)
        nc.vector.tensor_scalar(latn2[:, 256:512], latn[:], 0.0, None, op0=A.add)

        # A_i + b1 tables (bias columns for the relu epilogue)
        ABe = act.tile([128, 512], F32, tag="ABe")
        ABo = act.tile([128, 512], F32, tag="ABo")
        for (ab, w1, b1) in ((ABe, hw1e, hb1e), (ABo, hw1o, hb1o)):
            for hh in range(2):
                ps = pc_psum(128, N)
                nc.tensor.matmul(ps[:], w1[:, 0 * 256 + hh * 128: 0 * 256 + (hh + 1) * 128],
                                 latn2[:, 0:256], start=True, stop=True)
                nc.scalar.activation(ab[:, hh * 256:(hh + 1) * 256], ps[:],
                                     AF.Identity, bias=b1[:, hh:hh + 1])

        # op-head B term, precomputed to SBUF (added via DVE tt in the loop
        # instead of re-streaming W1r^T @ latn2 through PE every iteration)
        Bo = []
        for hh in range(2):
            ps = pc_psum(128, 512)
            nc.tensor.matmul(ps[:], hw1o[:, 1 * 256 + hh * 128: 1 * 256 + (hh + 1) * 128],
                             latn2[:], start=True, stop=True)
            bo = act.tile([128, 512], F32, tag=f"Bo{hh}")
            nc.scalar.activation(bo[:], ps[:], AF.Identity)
            Bo.append(bo)

        # query encoder + next_pred head
        qeT = act.tile([QE, N], F32, tag="qeT")
        ps = pc_psum(QE, N)
        nc.tensor.matmul(ps[:], qw[:], qT[:], start=True, stop=True)
        nc.scalar.activation(qeT[:], ps[:], AF.Relu, bias=qb[:])
        nh1T = act.tile([128, 512], F32, tag="nh1T")
        for hh in range(2):
            ps = pc_psum(128, N)
            nc.tensor.matmul(ps[:], nw1a[:, 0 * 256 + hh * 128: (0 * 256) + (hh + 1) * 128],
                             latn[:], start=True, stop=False)
            nc.tensor.matmul(ps[:], nw1a[:, 1 * 256 + hh * 128: (1 * 256) + (hh + 1) * 128],
                             nembT[:, 0:256], start=False, stop=False)
            nc.tensor.matmul(ps[:], nw1a[:, 2 * 256 + hh * 128: (2 * 256) + (hh + 1) * 128],
                             nembT[:, 256:512], start=False, stop=False)
            nc.tensor.matmul(ps[:], nw1b[:, hh * 128:(hh + 1) * 128], qeT[:],
                             start=False, stop=True)
            nc.scalar.activation(nh1T[:, hh * 256:(hh + 1) * 256], ps[:],
                                 AF.Relu, bias=nb1[:, hh:hh + 1])
        ps = ps2.tile([1, N], F32, tag="acc")
        for hh in range(2):
            nc.tensor.matmul(ps[:], nw2[:, hh:hh + 1], nh1T[:, hh * 256:(hh + 1) * 256],
                             start=(hh == 0), stop=(hh == 1))
        np_sb = wrk.tile([1, N], F32, tag="np_sb")
        nc.vector.tensor_scalar(np_sb[:], ps[:], nb2[0:1, 0:1], None, op0=A.add)
        nc.sync.dma_start(np_d.rearrange("(a b) -> a b", a=1), np_sb[:])

        # ---------------- main pair loop: 64 x (2 rows) ----------------
        # epilogue engine split; absd on DVE (sub f32r + int32 sign-clear)
        EPI_ACT = {(0, 0, 0), (0, 0, 1), (0, 1, 0), (0, 1, 1)}
        edge_acc = None
        op_acc = None

        def emit_absd(ip):
            t = wrk.tile([128, 512], F32R, tag="absd")
            for ii in range(2):
                nc.scalar.activation(t[:, ii * 256:(ii + 1) * 256], latn[:],
                                     AF.Abs, bias=latn[:, 2 * ip + ii:2 * ip + ii + 1],
                                     scale=-1.0)
            return t

        absd_next = emit_absd(0)
        for ip in range(NI // 2):
            i0 = 2 * ip
            te = ip % EBLK
            to = ip % OBLK
            if te == 0:
                edge_acc = ps2.tile([EBLK, 512], F32, tag="acc")
            if to == 0:
                op_acc = ps2.tile([32, 512], F32, tag="po")
            absd = absd_next
            if ip + 1 < NI // 2:
                absd_next = emit_absd(ip + 1)
            hts = {}
            for hd, (w1, ab) in enumerate(((hw1e, ABe), (hw1o, ABo))):
                for hh in range(2):
                    ps = ps1.tile([128, 512], F32, tag=PC_TAGS[hd * 2 + hh])
                    nc.tensor.matmul(ps[:], w1[:, 2 * 256 + hh * 128: 2 * 256 + (hh + 1) * 128],
                                     absd[:], start=True, stop=(hd == 1))
                    if hd == 0:
                        nc.tensor.matmul(ps[:], w1[:, 1 * 256 + hh * 128: 1 * 256 + (hh + 1) * 128],
                                         latn2[:], start=False, stop=True)
                    ht = wrk.tile([128, 512], F32R, tag=f"ht{hd}{hh}")
                    if hd == 1:
                        tmp = wrk.tile([128, 512], F32, tag=f"tmpo{hh}")
                        nc.vector.tensor_tensor(tmp[:], ps[:], Bo[hh][:], op=A.add)
                        iv_src = tmp
                    for ii in range(2):
                        ov = ht[:, ii * 256:(ii + 1) * 256]
                        bias = ab[:, hh * 256 + i0 + ii: hh * 256 + i0 + ii + 1]
                        if hd == 1:
                            nc.vector.tensor_scalar(ov, iv_src[:, ii * 256:(ii + 1) * 256],
                                                    bias, 0.0, op0=A.add, op1=A.max)
                        elif (hd, hh, ii) in EPI_ACT:
                            nc.scalar.activation(ov, ps[:, ii * 256:(ii + 1) * 256],
                                                 AF.Relu, bias=bias)
                        else:
                            nc.vector.tensor_scalar(ov, ps[:, ii * 256:(ii + 1) * 256],
                                                    bias, 0.0, op0=A.add, op1=A.max)
                    hts[(hd, hh)] = ht
            # edge head 2nd layer: accumulate into row te of edge_acc
            for hh in range(2):
                lw = ew2sh[:, (hh * EBLK + te) * EBLK:(hh * EBLK + te + 1) * EBLK]
                nc.tensor.matmul(edge_acc[:], lw, hts[(0, hh)][:],
                                 start=(te == 0 and hh == 0),
                                 stop=(te == EBLK - 1 and hh == 1),
                                 skip_group_check=True)
            # op head 2nd layer: accumulate into rows to*8..to*8+8
            for hh in range(2):
                lw = ow2sh[:, (hh * OBLK + to) * 32:(hh * OBLK + to + 1) * 32]
                nc.tensor.matmul(op_acc[:], lw, hts[(1, hh)][:],
                                 start=(to == 0 and hh == 0),
                                 stop=(to == OBLK - 1 and hh == 1),
                                 skip_group_check=True)
            # drain op block (bias via per-partition scalar)
            if to == OBLK - 1:
                op_blk = wrk.tile([32, 512], F32, tag="op_blk")
                nc.vector.tensor_scalar(op_blk[:], op_acc[:], colp[0:32, 0:1],
                                        None, op0=A.add)
                for t in range(OBLK):
                    nc.sync.dma_start(
                        op_d.rearrange("(npp t ii k) j -> npp t k ii j",
                                       t=OBLK, ii=2, k=NOPS)[ip // OBLK, t, :, :, :],
                        op_blk[t * NOPS:(t + 1) * NOPS, :]
                        .rearrange("k (ii j) -> k ii j", ii=2))
            # drain edge block (bias via per-partition scalar)
            if te == EBLK - 1:
                edge_blk = wrk.tile([EBLK, 512], F32, tag="edge_blk")
                nc.scalar.activation(edge_blk[:], edge_acc[:], AF.Identity,
                                     bias=colp[0:EBLK, 1:2])
                nc.sync.dma_start(
                    edge_d.rearrange("(npp t ii) j -> npp t ii j",
                                     t=EBLK, ii=2)[ip // EBLK, :, :, :],
                    edge_blk[:].rearrange("t (ii j) -> t ii j", ii=2))

    nc.compile()
    return nc


def _get_nc():
    if "nc" not in _NC_CACHE:
        _NC_CACHE["nc"] = _build()
    return _NC_CACHE["nc"]


TRACE = False
LAST_EXEC_NS = None


def kernel(**inputs):
    global LAST_EXEC_NS
    nc = _get_nc()
    w_names = [k for k in inputs if k not in ("obs", "latent", "query")]
    eh_w2 = np.asarray(inputs["eh_w2"], dtype=np.float32)
    oh_w2 = np.asarray(inputs["oh_w2"], dtype=np.float32)
    ew2sh = np.zeros((128, 2 * EBLK * EBLK), np.float32)
    for hh in range(2):
        for t in range(EBLK):
            ew2sh[:, (hh * EBLK + t) * EBLK + t] = eh_w2[hh * 128:(hh + 1) * 128, 0]
    ow2sh = np.zeros((128, 2 * OBLK * 32), np.float32)
    for hh in range(2):
        for t in range(OBLK):
            c = (hh * OBLK + t) * 32 + t * NOPS
            ow2sh[:, c:c + NOPS] = oh_w2[hh * 128:(hh + 1) * 128, :]
    col_pack = np.zeros((128, 2), np.float32)
    col_pack[0:32, 0] = np.tile(np.asarray(inputs["oh_b2"], np.float32), OBLK)
    col_pack[0:EBLK, 1] = np.asarray(inputs["eh_b2"], np.float32)[0]
    in_maps = []
    for c in range(N_CORES):
        b, half = c // 2, c % 2
        m = {k: np.ascontiguousarray(np.asarray(inputs[k], dtype=np.float32))
             for k in w_names}
        m["ew2sh_in"] = ew2sh
        m["ow2sh_in"] = ow2sh
        m["col_pack"] = col_pack
        for k in ("obs", "latent", "query"):
            arr = np.asarray(inputs[k][b], dtype=np.float32)
            if half:
                arr = np.roll(arr, -NI, axis=0)
            m[k + "_t"] = np.ascontiguousarray(arr.T)
        in_maps.append(m)

    res = run_bass_kernel_spmd(nc, in_maps, list(range(N_CORES)), trace=TRACE)
    LAST_EXEC_NS = res.exec_time_ns

    edge = np.empty((B, N, N), np.float32)
    op = np.empty((B, N, N, NOPS), np.float32)
    nxt = np.empty((B, N, 1), np.float32)
    for c in range(N_CORES):
        b, half = c // 2, c % 2
        r = res.results[c]
        e = r["edge_out"]                       # [128, 256] (j in rolled order)
        o = r["op_out"].reshape(NI, NOPS, N).transpose(0, 2, 1)  # [128, 256, 8]
        if half:
            e = np.roll(e, NI, axis=1)
            o = np.roll(o, NI, axis=1)
        rows = slice(half * NI, (half + 1) * NI)
        edge[b, rows, :] = e
        op[b, rows, :, :] = o
        nxt[b, rows, 0] = r["np_out"][:NI]
    idx = np.arange(N)
    edge[:, idx, idx] = -8.0
    return edge, op, nxt


if __name__ == "__main__":
    nc = _get_nc()
    print("built ok")


# revision 28
# speedup vs baseline: 1.1768x; 1.1768x over previous
"""Trainium2 Bass kernel for nn_PersistentObserver (GNN message passing).

Math (per batch item b, N=256 nodes):
  node_emb = relu(relu(obs@W1+b1)@W2+b2)            [N,256]
  upd      = node_emb@Wu+bu                         [N,128]
  lat      = GRUCell(upd, latent)                   [N,128]
  pair_ij  = [lat_i, lat_j, |lat_i-lat_j|]          [N,N,384]
  edge     = relu(pair@We1+be1)@We2+be2, diag=-8    [N,N]
  op       = relu(pair@Wo1+bo1)@Wo2+bo2             [N,N,8]
  next     = relu([lat,node_emb,q_emb]@Wn1+bn1)@Wn2+bn2  [N,1]

Key factorization: pair@W1 = A_i + B_j + |lat_i-lat_j|@W1a where
A = lat@W1_left, B = lat@W1_right depend on one index only. Only the
abs-diff term needs the N^2 matmul.

Sharding: 8 cores = 4 batches x 2 row-halves. Odd cores receive inputs
rolled by -128 along the node axis (the computation is permutation
equivariant), so every core runs the identical program computing rows
0..127; the host un-rolls the j axis on gather.

Layout: transposed ("T"): features on partitions, nodes on the free
axis. Hidden H=256 split in two partition halves. Inner loop handles
2 rows per iteration (free dim 512). Second-layer outputs accumulate
in PSUM across iterations using phase-shifted weight matrices (the w2
column placed at the iteration's output row, zeros elsewhere), so the
PSUM->SBUF->DRAM drain happens once per 16 (edge) / 4 (op) iterations;
output biases are folded into the drain copies as per-partition scalars.
B-term handling is asymmetric: the edge head accumulates B on the PE
(second matmul into PSUM); the op head adds a precomputed SBUF copy of B
via one DVE tensor_tensor, which lets its relu epilogue run as 2x-mode
SBUF-source tensor_scalars instead of 1x PSUM-source ops.
"""
import numpy as np
from contextlib import ExitStack

import concourse.bass as bass
import concourse.tile as tile
from concourse import bacc, mybir
from concourse.bass_utils import run_bass_kernel_spmd

F32 = mybir.dt.float32
F32R = mybir.dt.float32r
F16 = mybir.dt.float16
I32 = mybir.dt.int32
A = mybir.AluOpType
AF = mybir.ActivationFunctionType

B, N, OBS, QDIM = 4, 256, 64, 32
H, D = 256, 128
NOPS = 8
QE = 64          # H // 4
NI = 128         # i-rows per core
N_CORES = 8
EBLK = 16        # edge accumulation block (iterations)
OBLK = 4         # op accumulation block

_NC_CACHE = {}


def _build():
    nc = bacc.Bacc("TRN2", target_bir_lowering=False, debug=False,
                   num_devices=N_CORES)

    di = {}
    def inp(name, shape):
        di[name] = nc.dram_tensor(name, list(shape), F32, kind="ExternalInput").ap()
        return di[name]

    inp("enc_w1", (OBS, H)); inp("enc_b1", (H,))
    inp("enc_w2", (H, H)); inp("enc_b2", (H,))
    inp("upd_w", (H, D)); inp("upd_b", (D,))
    inp("gru_wi", (D, 3 * D)); inp("gru_bi", (3 * D,))
    inp("gru_wh", (D, 3 * D)); inp("gru_bh", (3 * D,))
    inp("eh_w1", (3 * D, H)); inp("eh_b1", (H,))
    inp("eh_w2", (H, 1)); inp("eh_b2", (1,))
    inp("oh_w1", (3 * D, H)); inp("oh_b1", (H,))
    inp("oh_w2", (H, NOPS)); inp("oh_b2", (NOPS,))
    inp("q_w", (QDIM, QE)); inp("q_b", (QE,))
    inp("nh_w1", (D + H + QE, H)); inp("nh_b1", (H,))
    inp("nh_w2", (H, 1)); inp("nh_b2", (1,))
    inp("obs_t", (OBS, N)); inp("latent_t", (D, N)); inp("query_t", (QDIM, N))
    inp("ew2sh_in", (128, 2 * EBLK * EBLK)); inp("ow2sh_in", (128, 2 * OBLK * 32))
    inp("col_pack", (128, 2))

    edge_d = nc.dram_tensor("edge_out", [NI, N], F32, kind="ExternalOutput").ap()
    op_d = nc.dram_tensor("op_out", [NI * NOPS, N], F32, kind="ExternalOutput").ap()
    np_d = nc.dram_tensor("np_out", [N], F32, kind="ExternalOutput").ap()

    with tile.TileContext(nc) as tc, ExitStack() as ctx:
        cst = ctx.enter_context(tc.tile_pool(name="cst", bufs=1))
        act = ctx.enter_context(tc.tile_pool(name="act", bufs=1))
        wrk = ctx.enter_context(tc.tile_pool(name="wrk", bufs=3))
        ps1 = ctx.enter_context(tc.tile_pool(name="ps1", bufs=1, space="PSUM"))
        ps1b = ctx.enter_context(tc.tile_pool(name="ps1b", bufs=2, space="PSUM"))
        ps2 = ctx.enter_context(tc.tile_pool(name="ps2", bufs=1, space="PSUM"))

        # ---------------- weight / bias loads ----------------
        def load(name, view, shape, dt=F32, eng=None):
            t = cst.tile(list(shape), dt, tag=name)
            src = view if dt == F32 else view.bitcast(F32R)
            (eng or nc.sync).dma_start(t[:], src)
            return t

        # critical-path loads first: activations + encoder/GRU weights
        obsT = load("obsT", di["obs_t"][:], (OBS, N))
        latT = load("latT", di["latent_t"][:], (D, N))
        qT = load("qT", di["query_t"][:], (QDIM, N))
        ew1 = load("ew1", di["enc_w1"][:], (OBS, H))
        eb1 = load("eb1", di["enc_b1"].rearrange("(s p) -> p s", s=2), (128, 2))
        ew2 = load("ew2", di["enc_w2"].rearrange("(kh kl) m -> kl kh m", kh=2), (128, 512))
        eb2 = load("eb2", di["enc_b2"].rearrange("(s p) -> p s", s=2), (128, 2))
        uw = load("uw", di["upd_w"].rearrange("(kh kl) m -> kl kh m", kh=2), (128, 256))
        ub = load("ub", di["upd_b"].rearrange("(p o) -> p o", o=1), (128, 1))
        gwi = load("gwi", di["gru_wi"][:], (128, 384))
        gbi = load("gbi", di["gru_bi"].rearrange("(s p) -> p s", s=3), (128, 3))
        gwh = load("gwh", di["gru_wh"][:], (128, 384))
        gbh = load("gbh", di["gru_bh"].rearrange("(s p) -> p s", s=3), (128, 3))
        hw1e = load("hw1e", di["eh_w1"].rearrange("(s kl) m -> kl s m", s=3), (128, 768), F32R, nc.gpsimd)
        hb1e = load("hb1e", di["eh_b1"].rearrange("(s p) -> p s", s=2), (128, 2))
        hw1o = load("hw1o", di["oh_w1"].rearrange("(s kl) m -> kl s m", s=3), (128, 768), F32R, nc.gpsimd)
        hb1o = load("hb1o", di["oh_b1"].rearrange("(s p) -> p s", s=2), (128, 2))
        qw = load("qw", di["q_w"][:], (QDIM, QE))
        qb = load("qb", di["q_b"].rearrange("(p o) -> p o", o=1), (QE, 1))
        nw1a = load("nw1a", di["nh_w1"][0:384, :].rearrange("(s kl) m -> kl s m", s=3), (128, 768), F32, nc.gpsimd)
        nw1b = load("nw1b", di["nh_w1"][384:448, :], (QE, H))
        nb1 = load("nb1", di["nh_b1"].rearrange("(s p) -> p s", s=2), (128, 2))
        nw2 = load("nw2", di["nh_w2"].rearrange("(s kl) m -> kl s m", s=2), (128, 2))
        nb2 = load("nb2", di["nh_b2"].rearrange("(p o) -> p o", o=1), (1, 1))

        # phase-shifted second-layer weights (host-packed) + drain biases
        ew2sh = load("ew2sh", di["ew2sh_in"][:], (128, 2 * EBLK * EBLK), F32R, nc.gpsimd)
        ow2sh = load("ow2sh", di["ow2sh_in"][:], (128, 2 * OBLK * 32), F32R, nc.gpsimd)
        colp = load("colp", di["col_pack"][:], (128, 2))

        # ---------------- per-batch precompute ----------------
        PC_TAGS = ["he0", "he1", "ho0", "ho1"]
        pc_i = [0]
        def pc_psum(p_dim, f_dim):
            t = ps1.tile([p_dim, f_dim], F32, tag=PC_TAGS[2 + pc_i[0] % 2])
            pc_i[0] += 1
            return t

        # encoder layer 1: h1T[hh] = relu(W1[:,hh]^T @ obsT + b1)
        h1T = act.tile([128, 512], F32, tag="h1T")
        for hh in range(2):
            ps = pc_psum(128, N)
            nc.tensor.matmul(ps[:], ew1[:, hh * 128:(hh + 1) * 128], obsT[:],
                             start=True, stop=True)
            nc.scalar.activation(h1T[:, hh * 256:(hh + 1) * 256], ps[:],
                                 AF.Relu, bias=eb1[:, hh:hh + 1])
        # encoder layer 2
        nembT = act.tile([128, 512], F32, tag="nembT")
        for hh in range(2):
            ps = pc_psum(128, N)
            for kh in range(2):
                nc.tensor.matmul(ps[:], ew2[:, kh * 256 + hh * 128: kh * 256 + (hh + 1) * 128],
                                 h1T[:, kh * 256:(kh + 1) * 256],
                                 start=(kh == 0), stop=(kh == 1))
            nc.scalar.activation(nembT[:, hh * 256:(hh + 1) * 256], ps[:],
                                 AF.Relu, bias=eb2[:, hh:hh + 1])
        # upd head
        updT = act.tile([D, N], F32, tag="updT")
        ps = pc_psum(D, N)
        for kh in range(2):
            nc.tensor.matmul(ps[:], uw[:, kh * 128:(kh + 1) * 128],
                             nembT[:, kh * 256:(kh + 1) * 256],
                             start=(kh == 0), stop=(kh == 1))
        nc.scalar.activation(updT[:], ps[:], AF.Identity, bias=ub[:])
        # GRU gates
        giT = act.tile([D, 768], F32, tag="giT")
        ghT = act.tile([D, 768], F32, tag="ghT")
        for g in range(3):
            ps = pc_psum(D, N)
            nc.tensor.matmul(ps[:], gwi[:, g * 128:(g + 1) * 128], updT[:],
                             start=True, stop=True)
            nc.scalar.activation(giT[:, g * 256:(g + 1) * 256], ps[:],
                                 AF.Identity, bias=gbi[:, g:g + 1])
            ps = pc_psum(D, N)
            nc.tensor.matmul(ps[:], gwh[:, g * 128:(g + 1) * 128], latT[:],
                             start=True, stop=True)
            nc.scalar.activation(ghT[:, g * 256:(g + 1) * 256], ps[:],
                                 AF.Identity, bias=gbh[:, g:g + 1])
        rT = act.tile([D, N], F32, tag="rT")
        nc.vector.tensor_add(rT[:], giT[:, 0:256], ghT[:, 0:256])
        nc.scalar.activation(rT[:], rT[:], AF.Sigmoid)
        zT = act.tile([D, N], F32, tag="zT")
        nc.vector.tensor_add(zT[:], giT[:, 256:512], ghT[:, 256:512])
        nc.scalar.activation(zT[:], zT[:], AF.Sigmoid)
        nT = act.tile([D, N], F32, tag="nT")
        nc.vector.tensor_mul(nT[:], rT[:], ghT[:, 512:768])
        nc.vector.tensor_add(nT[:], nT[:], giT[:, 512:768])
        nc.scalar.activation(nT[:], nT[:], AF.Tanh)
        # latn = n + z*(lat - n)
        latn = act.tile([D, N], F32, tag="latn")
        nc.vector.tensor_sub(latn[:], latT[:], nT[:])
        nc.vector.tensor_mul(latn[:], zT[:], latn[:])
        nc.vector.tensor_add(latn[:], latn[:], nT[:])
        # duplicated f32r copy [latn | latn] for the per-i B accumulation
        latn2 = act.tile([D, 512], F32R, tag="latn2")
        nc.vector.tensor_scalar(latn2[:, 0:256], latn[:], 0.0, None, op0=A.add)
        nc.vector.tensor_scalar(latn2[:, 256:512], latn[:], 0.0, None, op0=A.add)

        # A_i + b1 tables (bias columns for the relu epilogue)
        ABe = act.tile([128, 512], F32, tag="ABe")
        ABo = act.tile([128, 512], F32, tag="ABo")
        for (ab, w1, b1) in ((ABe, hw1e, hb1e), (ABo, hw1o, hb1o)):
            for hh in range(2):
                ps = pc_psum(128, N)
                nc.tensor.matmul(ps[:], w1[:, 0 * 256 + hh * 128: 0 * 256 + (hh + 1) * 128],
                                 latn2[:, 0:256], start=True, stop=True)
                nc.scalar.activation(ab[:, hh * 256:(hh + 1) * 256], ps[:],
                                     AF.Identity, bias=b1[:, hh:hh + 1])

        # op-head B term, precomputed to SBUF (added via DVE tt in the loop
        # instead of re-streaming W1r^T @ latn2 through PE every iteration)
        Bo = []
        for hh in range(2):
            ps = pc_psum(128, 512)
            nc.tensor.matmul(ps[:], hw1o[:, 1 * 256 + hh * 128: 1 * 256 + (hh + 1) * 128],
                             latn2[:], start=True, stop=True)
            bo = act.tile([128, 512], F32, tag=f"Bo{hh}")
            nc.scalar.activation(bo[:], ps[:], AF.Identity)
            Bo.append(bo)

        # query encoder + next_pred head
        qeT = act.tile([QE, N], F32, tag="qeT")
        ps = pc_psum(QE, N)
        nc.tensor.matmul(ps[:], qw[:], qT[:], start=True, stop=True)
        nc.scalar.activation(qeT[:], ps[:], AF.Relu, bias=qb[:])
        nh1T = act.tile([128, 512], F32, tag="nh1T")
        for hh in range(2):
            ps = pc_psum(128, N)
            nc.tensor.matmul(ps[:], nw1a[:, 0 * 256 + hh * 128: (0 * 256) + (hh + 1) * 128],
                             latn[:], start=True, stop=False)
            nc.tensor.matmul(ps[:], nw1a[:, 1 * 256 + hh * 128: (1 * 256) + (hh + 1) * 128],
                             nembT[:, 0:256], start=False, stop=False)
            nc.tensor.matmul(ps[:], nw1a[:, 2 * 256 + hh * 128: (2 * 256) + (hh + 1) * 128],
                             nembT[:, 256:512], start=False, stop=False)
            nc.tensor.matmul(ps[:], nw1b[:, hh * 128:(hh + 1) * 128], qeT[:],
                             start=False, stop=True)
            nc.scalar.activation(nh1T[:, hh * 256:(hh + 1) * 256], ps[:],
                                 AF.Relu, bias=nb1[:, hh:hh + 1])
        ps = ps2.tile([1, N], F32, tag="acc")
        for hh in range(2):
            nc.tensor.matmul(ps[:], nw2[:, hh:hh + 1], nh1T[:, hh * 256:(hh + 1) * 256],
                             start=(hh == 0), stop=(hh == 1))
        np_sb = wrk.tile([1, N], F32, tag="np_sb")
        nc.vector.tensor_scalar(np_sb[:], ps[:], nb2[0:1, 0:1], None, op0=A.add)
        nc.sync.dma_start(np_d.rearrange("(a b) -> a b", a=1), np_sb[:])

        # ---------------- main pair loop: 64 x (2 rows) ----------------
        # epilogue engine split; absd on DVE (sub f32r + int32 sign-clear)
        EPI_ACT = {(0, 0, 0), (0, 0, 1), (0, 1, 0), (0, 1, 1)}
        edge_acc = None
        op_acc = None

        def emit_absd(ip):
            t = wrk.tile([128, 512], F32R, tag="absd")
            for ii in range(2):
                nc.scalar.activation(t[:, ii * 256:(ii + 1) * 256], latn[:],
                                     AF.Abs, bias=latn[:, 2 * ip + ii:2 * ip + ii + 1],
                                     scale=-1.0)
            return t

        absd_next = emit_absd(0)
        for ip in range(NI // 2):
            i0 = 2 * ip
            te = ip % EBLK
            to = ip % OBLK
            if te == 0:
                edge_acc = ps2.tile([EBLK, 512], F32, tag="acc")
            if to == 0:
                op_acc = ps2.tile([32, 512], F32, tag="po")
            absd = absd_next
            if ip + 1 < NI // 2:
                absd_next = emit_absd(ip + 1)
            hts = {}
            for hd, (w1, ab) in enumerate(((hw1e, ABe), (hw1o, ABo))):
                for hh in range(2):
                    ps = (ps1b if hd == 0 else ps1).tile([128, 512], F32,
                                                         tag=PC_TAGS[hd * 2 + hh])
                    nc.tensor.matmul(ps[:], w1[:, 2 * 256 + hh * 128: 2 * 256 + (hh + 1) * 128],
                                     absd[:], start=True, stop=(hd == 1))
                    if hd == 0:
                        nc.tensor.matmul(ps[:], w1[:, 1 * 256 + hh * 128: 1 * 256 + (hh + 1) * 128],
                                         latn2[:], start=False, stop=True)
                    ht = wrk.tile([128, 512], F32R, tag=f"ht{hd}{hh}")
                    if hd == 1:
                        tmp = wrk.tile([128, 512], F32, tag=f"tmpo{hh}")
                        nc.vector.tensor_tensor(tmp[:], ps[:], Bo[hh][:], op=A.add)
                        iv_src = tmp
                    for ii in range(2):
                        ov = ht[:, ii * 256:(ii + 1) * 256]
                        bias = ab[:, hh * 256 + i0 + ii: hh * 256 + i0 + ii + 1]
                        if hd == 1:
                            nc.vector.tensor_scalar(ov, iv_src[:, ii * 256:(ii + 1) * 256],
                                                    bias, 0.0, op0=A.add, op1=A.max)
                        elif (hd, hh, ii) in EPI_ACT:
                            nc.scalar.activation(ov, ps[:, ii * 256:(ii + 1) * 256],
                                                 AF.Relu, bias=bias)
                        else:
                            nc.vector.tensor_scalar(ov, ps[:, ii * 256:(ii + 1) * 256],
                                                    bias, 0.0, op0=A.add, op1=A.max)
                    hts[(hd, hh)] = ht
            # edge head 2nd layer: accumulate into row te of edge_acc
            for hh in range(2):
                lw = ew2sh[:, (hh * EBLK + te) * EBLK:(hh * EBLK + te + 1) * EBLK]
                nc.tensor.matmul(edge_acc[:], lw, hts[(0, hh)][:],
                                 start=(te == 0 and hh == 0),
                                 stop=(te == EBLK - 1 and hh == 1),
                                 skip_group_check=True)
            # op head 2nd layer: accumulate into rows to*8..to*8+8
            for hh in range(2):
                lw = ow2sh[:, (hh * OBLK + to) * 32:(hh * OBLK + to + 1) * 32]
                nc.tensor.matmul(op_acc[:], lw, hts[(1, hh)][:],
                                 start=(to == 0 and hh == 0),
                                 stop=(to == OBLK - 1 and hh == 1),
                                 skip_group_check=True)
            # drain op block (bias via per-partition scalar)
            if to == OBLK - 1:
                op_blk = wrk.tile([32, 512], F32, tag="op_blk")
                nc.vector.tensor_scalar(op_blk[:], op_acc[:], colp[0:32, 0:1],
                                        None, op0=A.add)
                for t in range(OBLK):
                    nc.sync.dma_start(
                        op_d.rearrange("(npp t ii k) j -> npp t k ii j",
                                       t=OBLK, ii=2, k=NOPS)[ip // OBLK, t, :, :, :],
                        op_blk[t * NOPS:(t + 1) * NOPS, :]
                        .rearrange("k (ii j) -> k ii j", ii=2))
            # drain edge block (bias via per-partition scalar)
            if te == EBLK - 1:
                edge_blk = wrk.tile([EBLK, 512], F32, tag="edge_blk")
                nc.scalar.activation(edge_blk[:], edge_acc[:], AF.Identity,
                                     bias=colp[0:EBLK, 1:2])
                nc.sync.dma_start(
                    edge_d.rearrange("(npp t ii) j -> npp t ii j",
                                     t=EBLK, ii=2)[ip // EBLK, :, :, :],
                    edge_blk[:].rearrange("t (ii j) -> t ii j", ii=2))

    nc.compile()
    return nc


def _get_nc():
    if "nc" not in _NC_CACHE:
        _NC_CACHE["nc"] = _build()
    return _NC_CACHE["nc"]


TRACE = False
LAST_EXEC_NS = None


def kernel(**inputs):
    global LAST_EXEC_NS
    nc = _get_nc()
    w_names = [k for k in inputs if k not in ("obs", "latent", "query")]
    eh_w2 = np.asarray(inputs["eh_w2"], dtype=np.float32)
    oh_w2 = np.asarray(inputs["oh_w2"], dtype=np.float32)
    ew2sh = np.zeros((128, 2 * EBLK * EBLK), np.float32)
    for hh in range(2):
        for t in range(EBLK):
            ew2sh[:, (hh * EBLK + t) * EBLK + t] = eh_w2[hh * 128:(hh + 1) * 128, 0]
    ow2sh = np.zeros((128, 2 * OBLK * 32), np.float32)
    for hh in range(2):
        for t in range(OBLK):
            c = (hh * OBLK + t) * 32 + t * NOPS
            ow2sh[:, c:c + NOPS] = oh_w2[hh * 128:(hh + 1) * 128, :]
    col_pack = np.zeros((128, 2), np.float32)
    col_pack[0:32, 0] = np.tile(np.asarray(inputs["oh_b2"], np.float32), OBLK)
    col_pack[0:EBLK, 1] = np.asarray(inputs["eh_b2"], np.float32)[0]
    in_maps = []
    for c in range(N_CORES):
        b, half = c // 2, c % 2
        m = {k: np.ascontiguousarray(np.asarray(inputs[k], dtype=np.float32))
             for k in w_names}
        m["ew2sh_in"] = ew2sh
        m["ow2sh_in"] = ow2sh
        m["col_pack"] = col_pack
        for k in ("obs", "latent", "query"):
            arr = np.asarray(inputs[k][b], dtype=np.float32)
            if half:
                arr = np.roll(arr, -NI, axis=0)
            m[k + "_t"] = np.ascontiguousarray(arr.T)
        in_maps.append(m)

    res = run_bass_kernel_spmd(nc, in_maps, list(range(N_CORES)), trace=TRACE)
    LAST_EXEC_NS = res.exec_time_ns

    edge = np.empty((B, N, N), np.float32)
    op = np.empty((B, N, N, NOPS), np.float32)
    nxt = np.empty((B, N, 1), np.float32)
    for c in range(N_CORES):
        b, half = c // 2, c % 2
        r = res.results[c]
        e = r["edge_out"]                       # [128, 256] (j in rolled order)
        o = r["op_out"].reshape(NI, NOPS, N).transpose(0, 2, 1)  # [128, 256, 8]
        if half:
            e = np.roll(e, NI, axis=1)
            o = np.roll(o, NI, axis=1)
        rows = slice(half * NI, (half + 1) * NI)
        edge[b, rows, :] = e
        op[b, rows, :, :] = o
        nxt[b, rows, 0] = r["np_out"][:NI]
    idx = np.arange(N)
    edge[:, idx, idx] = -8.0
    return edge, op, nxt


if __name__ == "__main__":
    nc = _get_nc()
    print("built ok")
